# revision 27
# baseline (speedup 1.0000x reference)
"""Trainium2 Bass kernel for nn_ARIG_Fusion (dual sigmoid gating + proj + BatchNorm + LIF).

Strategy (8 NeuronCores, SPMD):
  - Shard batch B=32 into 8 shards of 4. Each core handles rows (t, b_loc, n)
    = 4*4*1024 = 16384 rows of C=256 channels.
  - All tensors live on-chip in TRANSPOSED layout [channel, row]; the host
    pre-transposes inputs and post-transposes outputs.
  - Inputs stay f32 in HBM/SBUF but are DECLARED f32r, so the gate GEMMs
    consume them directly (f32r = same bits, 1 cyc/row on the PE vs 4 for
    fp32). The projection GEMM also runs f32r.
  - b_proj is dropped on device: BatchNorm subtracts the per-channel mean,
    so a per-channel bias before BN cancels exactly.
  - outb is stored as int16 (Act Copy with scale=QS): absolute quantization
    error (1/2QS ~ 4e-5) beats fp16's relative error ~8x and halves SBUF.
  - BatchNorm stats: per-tile bn_stats on the int16 outb -> bn_aggr, then a
    [128,4] AllReduce across 8 cores combines (mean, E[x^2]) in quant units.
  - The stats barrier serializes two phases, so each phase is balanced
    across engines. Engine facts (cost model): Pool elementwise is Q7
    software at ~2 ns/row + 95 ns/op and supports only TensorTensor and
    memset (no TensorScalarPtr); DVE TT is ~1.09 ns/row, DVE tensor_scalar
    is ~0.55 (2x mode, all-SBUF); ACT is ~0.89 ns/row incl. overhead.
  - Gating: v1 on DVE; v2/vr-add split DVE/Pool by a column fraction;
    bn_stats on DVE; outb copy split ACT/DVE. The ~33 MB input DMA is the
    phase floor.
  - LIF (T=4, rescaled state U_t = v_t/(1-tau)^t): per (t, q-chunk, j-half)
    ACT dequantizes + applies the BN affine (ay = asc_t*outb + ash_t), then
    the (q,j) recurrence chain runs ENGINE-LOCAL (no cross-engine hops in
    the scan): U = ay + W; s' = (U < th_t) u8 (inverted spike -> the host
    emits 1 - s'); W = U * s' (hard reset). 6 chains on DVE (immediate
    thresholds, is_lt in 2x mode) + 2 chains on Pool (is_lt as TT against
    memset threshold tiles).
"""

import math

import numpy as np

T, B, N, C = 4, 32, 1024, 256
NCORES = 8
BL = B // NCORES          # 4 batches per core
R = T * BL * N            # 16384 rows per core
RT = BL * N               # 4096 rows per t-slice
F = 512                   # gating tile columns
NT = R // F               # 32 gating tiles
CC = 1024                 # LIF column chunk (nq = RT // CC)
EPS = 1e-5
V_TH = 1.0
QS = 12288.0              # outb int16 quantization scale (range ~±2.67)

_program_cache = {}

# tuning knobs (read at trace time)
GIN_BUFS = 3
GATE_BUFS = 2
V_BUFS = 2
PG_BUFS = 1
PO_BUFS = 2
LIF_BUFS = 3
AY_BUFS = 4
S_BUFS = 12
STAGGER = 1              # software-pipeline proj/copy/bn by 1-2 tiles
V2_POOL_COLS = 1024      # cols of v2 (of 2*F=1024) computed on Pool
ADD_POOL_COLS = 256       # cols of vr add computed on Pool
COPY_DVE_COLS = 0        # cols of outb copy (of 1024) done on DVE
COPY_POOL_COLS = 0       # cols of outb copy done on Pool (TT vs QS tile)
N_POOL_HALVES = 7        # LIF chains (of 16) whose add/mult run on Pool
IN_DMA_QUEUES = 2        # spread xt input DMAs across sync/ACT queues


def _build_program(tau_inv: float, reps: int = 1, single_core: bool = False):
    cc = CC
    nq = RT // cc
    import concourse.bacc as bacc
    import concourse.bass as bass
    import concourse.tile as tile
    from concourse import mybir

    f32 = mybir.dt.float32
    f32r = mybir.dt.float32r
    i16 = mybir.dt.int16
    u8 = mybir.dt.uint8
    Alu = mybir.AluOpType
    Act = mybir.ActivationFunctionType

    # LIF rescaling: U_t = v_t / (1-tau)^t;  U_t = W_{t-1} + alpha_t * y_t
    one_m = 1.0 - tau_inv
    alphas = [tau_inv / (one_m ** t) for t in range(T)]
    ths = [V_TH / (one_m ** t) for t in range(T)]

    # LIF chains: state is elementwise, so the (q, j) scan splits freely by
    # column range into uniform half-chains of hc columns; the last
    # N_POOL_HALVES run their compare/mult on Pool, the rest on DVE.
    hc = cc // 2
    qjs = [(q, j) for j in (0, 1) for q in range(nq)]
    nchain = 2 * len(qjs)

    nc = bacc.Bacc("TRN2", target_bir_lowering=False, debug=False,
                   num_devices=1 if single_core else NCORES)

    xt_d = nc.dram_tensor("xt", [2, 2, 128, R], f32r, kind="ExternalInput")
    w1_d = nc.dram_tensor("w1", [128, 2, 2, 128], f32r, kind="ExternalInput")
    w2_d = nc.dram_tensor("w2", [128, 2, 2, 128], f32r, kind="ExternalInput")
    w3_d = nc.dram_tensor("w3", [128, 1, 2, 2, 128], f32r,
                          kind="ExternalInput")
    pp_d = nc.dram_tensor("pp", [128, 10], f32, kind="ExternalInput")
    bm_d = nc.dram_tensor("bm", [1, 2, 2, 128], f32r, kind="ExternalInput")
    on_d = nc.dram_tensor("on", [1, F], f32r, kind="ExternalInput")
    sp_d = nc.dram_tensor("sp", [2, 128, R], u8, kind="ExternalOutput")

    with tile.TileContext(nc) as tc:
      for _rep in range(reps):
        with tc.tile_pool(name="singles", bufs=1) as singles:
            w1s = singles.tile([128, 2, 2, 128], f32r)
            w2s = singles.tile([128, 2, 2, 128], f32r)
            w3s = singles.tile([128, 1, 2, 2, 128], f32r)
            pps = singles.tile([128, 10], f32)
            outb = singles.tile([128, 2, R], i16)
            stb = singles.tile([128, 2, NT, 6], f32)
            nc.sync.dma_start(w1s[:], w1_d[:, :, :, :])
            nc.sync.dma_start(w2s[:], w2_d[:, :, :, :])
            nc.sync.dma_start(w3s[:], w3_d[:, :, :, :, :])
            nc.sync.dma_start(pps[:], pp_d[:, :])
            bms = singles.tile([1, 2, 2, 128], f32r)
            ons = singles.tile([1, F], f32r)
            nc.sync.dma_start(bms[:], bm_d[:, :, :, :])
            nc.sync.dma_start(ons[:], on_d[:, :])
            if COPY_POOL_COLS > 0:
                qst = singles.tile([128, COPY_POOL_COLS], f32)
                nc.gpsimd.memset(qst[:], QS)

            # ---------------- gating + projection + stats ----------------
            with (
                tc.tile_pool(name="gin", bufs=GIN_BUFS) as gin,
                tc.tile_pool(name="gate", bufs=GATE_BUFS) as gatep,
                tc.tile_pool(name="vp", bufs=V_BUFS) as vp,
                tc.tile_pool(name="pg", bufs=PG_BUFS, space="PSUM") as pg,
                tc.tile_pool(name="po", bufs=PO_BUFS, space="PSUM") as po,
            ):
                # Software-pipelined: proj matmuls, the outb copy, and
                # bn_stats for tile k are emitted in LATER iterations so no
                # engine head-of-line blocks on a same-tile dependency.
                def emit_proj(k, vrk):
                    p3 = po.tile([128, 2, F], f32, tag="o", name="po")
                    for j in (0, 1):
                        nc.tensor.matmul(p3[:, j, :], w3s[:, 0, 0, j, :],
                                         vrk[:, 0, :], start=True, stop=False)
                        nc.tensor.matmul(p3[:, j, :], w3s[:, 0, 1, j, :],
                                         vrk[:, 1, :], start=False, stop=True)
                    return p3

                def emit_copy(k, p3k):
                    # quantize to int16 (scale QS); b_proj dropped (BN cancels)
                    # 3-way col split: ACT (Act.Copy), DVE (tensor_scalar),
                    # Pool (TT mult against the QS-memset tile)
                    cd, cp = COPY_DVE_COLS, COPY_POOL_COLS
                    ca = 2 * F - cd - cp
                    assert ca >= F, "ACT must cover at least the j=0 half"
                    k0 = k * F
                    nc.scalar.activation(outb[:, 0, k0:k0 + F], p3k[:, 0, :],
                                         Act.Copy, scale=QS)
                    e1 = ca - F   # ACT cols within j=1
                    if e1 > 0:
                        nc.scalar.activation(outb[:, 1, k0:k0 + e1],
                                             p3k[:, 1, :e1], Act.Copy,
                                             scale=QS)
                    if cd > 0:
                        nc.vector.tensor_scalar(outb[:, 1, k0 + e1:k0 + e1 + cd],
                                                p3k[:, 1, e1:e1 + cd], QS,
                                                None, Alu.mult)
                    if cp > 0:
                        nc.gpsimd.tensor_mul(outb[:, 1, k0 + e1 + cd:k0 + F],
                                             p3k[:, 1, e1 + cd:], qst[:, :cp])

                def emit_bn(k):
                    slk = slice(k * F, (k + 1) * F)
                    for j in (0, 1):
                        nc.vector.bn_stats(stb[:, j, k, :], outb[:, j, slk])

                vr_q = [None, None]   # vr tiles awaiting proj
                p3_q = [None, None]   # p3 tiles awaiting copy
                dma_engs = ([nc.sync, nc.scalar] if IN_DMA_QUEUES == 2
                            else [nc.sync])
                for i in range(NT):
                    sl = slice(i * F, (i + 1) * F)
                    xt = gin.tile([128, 2, 2, F], f32r, tag="xt", name="xt")
                    dap = xt_d[:, :, :, sl]
                    dma_engs[i % len(dma_engs)].dma_start(
                        xt[:], bass.AP(tensor=dap.tensor,
                                       offset=dap.offset,
                                       ap=[dap.ap[2], dap.ap[0],
                                           dap.ap[1], dap.ap[3]]))
                    a2 = xt[:, 0]
                    l2 = xt[:, 1]
                    g1 = pg.tile([128, 2, F], f32, tag="g1", name="g1")
                    g2 = pg.tile([128, 2, F], f32, tag="g2", name="g2")
                    for j in (0, 1):
                        nc.tensor.matmul(g1[:, j, :], bms[:, 0, j, :],
                                         ons[:, :], start=True, stop=False)
                        nc.tensor.matmul(g1[:, j, :], w1s[:, 0, j, :],
                                         a2[:, 0, :], start=False, stop=False)
                        nc.tensor.matmul(g1[:, j, :], w1s[:, 1, j, :],
                                         a2[:, 1, :], start=False, stop=True)
                    if STAGGER and i > 0:
                        p3_q[1] = emit_proj(i - 1, vr_q[1])
                    for j in (0, 1):
                        nc.tensor.matmul(g2[:, j, :], bms[:, 1, j, :],
                                         ons[:, :], start=True, stop=False)
                        nc.tensor.matmul(g2[:, j, :], w2s[:, 0, j, :],
                                         l2[:, 0, :], start=False, stop=False)
                        nc.tensor.matmul(g2[:, j, :], w2s[:, 1, j, :],
                                         l2[:, 1, :], start=False, stop=True)
                    gL = gatep.tile([128, 2, F], f32, tag="gL", name="gL")
                    gA = gatep.tile([128, 2, F], f32, tag="gA", name="gA")
                    nc.scalar.activation(gL[:], g1[:], Act.Sigmoid)
                    nc.scalar.activation(gA[:], g2[:], Act.Sigmoid)
                    if STAGGER and i > 0:
                        emit_copy(i - 1, p3_q[1])
                    if STAGGER and i > 1:
                        emit_bn(i - 2)
                    v1 = vp.tile([128, 2, F], f32, tag="v1", name="v1")
                    v2 = vp.tile([128, 2, F], f32, tag="v2", name="v2")
                    vr = vp.tile([128, 2, F], f32r, tag="vr", name="vr")
                    a2f = a2.bitcast(f32)
                    l2f = l2.bitcast(f32)
                    nc.vector.tensor_mul(v1[:], a2f, gA[:])
                    # col-split v2 / add across DVE and Pool (per j half)
                    vp_c = V2_POOL_COLS // 2
                    if vp_c > 0:
                        nc.gpsimd.tensor_mul(v2[:, :, F - vp_c:],
                                             l2f[:, :, F - vp_c:],
                                             gL[:, :, F - vp_c:])
                    if vp_c < F:
                        nc.vector.tensor_mul(v2[:, :, :F - vp_c],
                                             l2f[:, :, :F - vp_c],
                                             gL[:, :, :F - vp_c])
                    ap_c = ADD_POOL_COLS // 2
                    if ap_c > 0:
                        nc.gpsimd.tensor_add(vr[:, :, F - ap_c:],
                                             v1[:, :, F - ap_c:],
                                             v2[:, :, F - ap_c:])
                    if ap_c < F:
                        nc.vector.tensor_add(vr[:, :, :F - ap_c],
                                             v1[:, :, :F - ap_c],
                                             v2[:, :, :F - ap_c])
                    vr_q[1] = vr
                    if not STAGGER:
                        p3 = emit_proj(i, vr_q[1])
                        emit_copy(i, p3)
                        emit_bn(i)
                if STAGGER:
                    # drain: last tile's proj/copy/bn
                    p3_last = emit_proj(NT - 1, vr_q[1])
                    emit_copy(NT - 1, p3_last)
                    emit_bn(NT - 2)
                    emit_bn(NT - 1)
            # ---------------- stats finalize + all-reduce ----------------
            with (
                tc.tile_pool(name="fin", bufs=1) as fin,
                tc.tile_pool(name="dramp", bufs=1, space="DRAM") as dramp,
            ):
                mv = fin.tile([128, 2, 2], f32)
                ccs = fin.tile([128, 4], f32)
                for j in (0, 1):
                    nc.vector.bn_aggr(mv[:, j, :], stb[:, j, :, :])
                    nc.vector.tensor_copy(ccs[:, 2 * j:2 * j + 1], mv[:, j, 0:1])
                    nc.vector.tensor_mul(ccs[:, 2 * j + 1:2 * j + 2],
                                         mv[:, j, 0:1], mv[:, j, 0:1])
                    nc.vector.tensor_add(ccs[:, 2 * j + 1:2 * j + 2],
                                         ccs[:, 2 * j + 1:2 * j + 2],
                                         mv[:, j, 1:2])
                if single_core:
                    cg = fin.tile([128, 4], f32)
                    nc.vector.tensor_scalar(cg[:], ccs[:], float(NCORES),
                                            None, Alu.mult)
                else:
                    cc_in = dramp.tile([128, 4], f32)
                    cc_out = dramp.tile([128, 4], f32)
                    nc.gpsimd.dma_start(cc_in[:], ccs[:])
                    nc.gpsimd.collective_compute(
                        "AllReduce", Alu.add,
                        replica_groups=[list(range(NCORES))],
                        ins=[cc_in.opt()], outs=[cc_out.opt()],
                    )
                    cg = fin.tile([128, 4], f32)
                    nc.gpsimd.dma_start(cg[:], cc_out[:])

                mean = fin.tile([128, 2], f32)
                varp = fin.tile([128, 2], f32)
                sc = fin.tile([128, 2], f32)
                sh = fin.tile([128, 2], f32)
                t1 = fin.tile([128, 2], f32)
                t2 = fin.tile([128, 2], f32)
                r0 = fin.tile([128, 2], f32)
                for j in (0, 1):
                    jm = slice(j, j + 1)
                    nc.vector.tensor_scalar(mean[:, jm], cg[:, 2 * j:2 * j + 1],
                                            1.0 / NCORES, None, Alu.mult)
                    # varp = E[x^2] - mean^2 + eps  (in quant units: eps*QS^2)
                    nc.vector.tensor_scalar(varp[:, jm],
                                            cg[:, 2 * j + 1:2 * j + 2],
                                            1.0 / NCORES, None, Alu.mult)
                    nc.vector.tensor_mul(t1[:, jm], mean[:, jm], mean[:, jm])
                    nc.vector.tensor_sub(varp[:, jm], varp[:, jm], t1[:, jm])
                    nc.vector.tensor_scalar(varp[:, jm], varp[:, jm],
                                            EPS * QS * QS, None, Alu.add)
                # r0 = 1/sqrt(varp), via ACT sqrt + reciprocal + 2 Newton steps
                nc.scalar.activation(r0[:], varp[:], Act.Sqrt)
                nc.vector.reciprocal(r0[:], r0[:])
                for _ in range(2):
                    nc.vector.tensor_mul(t1[:], r0[:], r0[:])
                    nc.vector.tensor_mul(t2[:], t1[:], varp[:])
                    nc.vector.tensor_scalar(t2[:], t2[:], -0.5, 1.5,
                                            Alu.mult, Alu.add)
                    nc.vector.tensor_mul(r0[:], r0[:], t2[:])
                # sc = gamma * r0 (y = sc*outb_q + sh with outb_q in quant
                # units; the QS scale folds into r0 automatically)
                for j in (0, 1):
                    jm = slice(j, j + 1)
                    nc.vector.tensor_mul(sc[:, jm], pps[:, 6 + j:7 + j],
                                         r0[:, jm])
                    nc.vector.tensor_mul(t1[:, jm], mean[:, jm], sc[:, jm])
                    nc.vector.tensor_sub(sh[:, jm], pps[:, 8 + j:9 + j],
                                         t1[:, jm])

                # per-t pre-scaled BN affine: ay_t = alpha_t*(sc*outb+sh)
                asc = fin.tile([128, T, 2], f32)
                ash = fin.tile([128, T, 2], f32)
                for t in range(T):
                    for j in (0, 1):
                        nc.vector.tensor_scalar(asc[:, t, j:j + 1], sc[:, j:j + 1],
                                                alphas[t], None, Alu.mult)
                        nc.vector.tensor_scalar(ash[:, t, j:j + 1], sh[:, j:j + 1],
                                                alphas[t], None, Alu.mult)

                # ---------------- LIF scan + spike output ----------------
                # s' = (U < th) is the INVERTED spike: W = U*s' gives the
                # hard reset, and the host emits spikes = 1 - s'.
                # Each chain (q, j, col-range) runs its compare/mult on ONE
                # engine; with LIF_PE_ADD the U=ay+W add runs on the (idle)
                # PE as two identity matmuls accumulating into PSUM.
                # t-outer, op-type-grouped: per t-row each engine emits
                # all its adds, then compares, then resets - consecutive ops
                # on an engine belong to different chains, so dependent-op
                # latency is hidden. ALL compares run on DVE (tensor_scalar
                # u8-out is 2x there; Pool cannot emit u8 from f32): Pool
                # owns add/mult for the last N_POOL_HALVES chains (f32 TT).
                # State lives IN-PLACE in the ay tile: ay += W_prev;
                # s' = (ay < th) [DVE]; ay *= s' (ay then IS W for t+1).
                # Pool-owned chains' compares are emitted first so Pool's
                # resets unblock early.
                pool_own = set(range(nchain - N_POOL_HALVES, nchain))

                def chain_slice(ci, t):
                    qi, half = divmod(ci, 2)
                    q, j = qjs[qi]
                    c0 = t * RT + q * cc + half * hc
                    return j, slice(c0, c0 + hc)

                def ceng(ci):
                    return nc.gpsimd if ci in pool_own else nc.vector

                cmp_order = (list(range(nchain - N_POOL_HALVES, nchain))
                             + list(range(nchain - N_POOL_HALVES)))
                with tc.tile_pool(name="lif", bufs=LIF_BUFS) as lifp:
                    ayt = {}
                    prev = {}
                    for t in range(T):
                        for ci in range(nchain):
                            en = "p" if ci in pool_own else "d"
                            nch = (N_POOL_HALVES if ci in pool_own
                                   else nchain - N_POOL_HALVES)
                            j, csl = chain_slice(ci, t)
                            ay = lifp.tile([128, hc], f32, tag=f"ay{en}",
                                           name=f"ay{en}", bufs=2 * nch + 2)
                            nc.scalar.activation(
                                ay[:], outb[:, j, csl], Act.Identity,
                                bias=ash[:, t, j:j + 1],
                                scale=asc[:, t, j:j + 1])
                            prev[ci] = ayt.get(ci)
                            ayt[ci] = ay
                        if t > 0:
                            for ci in list(range(nchain - N_POOL_HALVES,
                                                 nchain))                                     + list(range(nchain - N_POOL_HALVES)):
                                ceng(ci).tensor_add(ayt[ci][:], ayt[ci][:],
                                                    prev[ci][:])
                        st = {}
                        for ci in cmp_order:
                            s = lifp.tile([128, hc], u8, tag="s", name="s",
                                          bufs=S_BUFS)
                            nc.vector.tensor_scalar(s[:], ayt[ci][:],
                                                    float(ths[t]), None,
                                                    Alu.is_lt)
                            st[ci] = s
                            j, csl = chain_slice(ci, t)
                            nc.sync.dma_start(sp_d[j, :, csl], s[:])
                        if t < T - 1:
                            for ci in cmp_order:
                                ceng(ci).tensor_mul(ayt[ci][:], ayt[ci][:],
                                                    st[ci][:])

    nc.compile()
    return nc


def _get_program(tau_inv: float, reps: int = 1, single_core: bool = False):
    key = (round(float(tau_inv), 12), reps, single_core)
    if key not in _program_cache:
        _program_cache[key] = _build_program(float(tau_inv), reps, single_core)
    return _program_cache[key]


def _shard_transpose(x):
    # [T,B,N,C] -> [cores, 2, 128, R] with rows ordered (t, b_loc, n)
    v = x.reshape(T, NCORES, BL, N, C)
    v = np.transpose(v, (1, 4, 0, 2, 3))
    return np.ascontiguousarray(v).reshape(NCORES, 2, 128, R)


def _prep_w(w):
    # lhsT chunks [p, k, j, q]: W.T viewed as [k,128p][j,128q]
    wt = np.ascontiguousarray(w.T).reshape(2, 128, 2, 128)
    return np.ascontiguousarray(wt.transpose(1, 0, 2, 3))


def _two(vec):
    return np.ascontiguousarray(vec.reshape(2, 128).T)


def _make_in_maps(inputs):
    x_attn = np.asarray(inputs["x_attn"], dtype=np.float32)
    x_lsm = np.asarray(inputs["x_lsm"], dtype=np.float32)
    at = _shard_transpose(x_attn)
    lt = _shard_transpose(x_lsm)
    xt = np.stack([at, lt], axis=1)  # [cores, 2, 2, 128, R]
    w1 = _prep_w(np.asarray(inputs["W_att"], dtype=np.float32))
    w2 = _prep_w(np.asarray(inputs["W_lsm"], dtype=np.float32))
    w3 = _prep_w(np.asarray(inputs["W_proj"], dtype=np.float32))[:, None]
    pp = np.concatenate(
        [_two(np.asarray(inputs["b_att"], dtype=np.float32)),
         _two(np.asarray(inputs["b_lsm"], dtype=np.float32)),
         _two(np.asarray(inputs["b_proj"], dtype=np.float32)),
         _two(np.asarray(inputs["gamma"], dtype=np.float32)),
         _two(np.asarray(inputs["beta"], dtype=np.float32))],
        axis=1)
    bm = np.ascontiguousarray(np.stack([
        np.asarray(inputs["b_att"], dtype=np.float32).reshape(2, 128),
        np.asarray(inputs["b_lsm"], dtype=np.float32).reshape(2, 128),
    ])[None])  # [1, 2, 2, 128]
    base = {"w1": w1, "w2": w2, "w3": w3, "pp": pp, "bm": bm,
            "on": np.ones((1, F), dtype=np.float32)}
    return [dict(base, xt=xt[s]) for s in range(NCORES)]


def kernel(**inputs):
    from concourse.bass_utils import run_bass_kernel_spmd

    lif_w = float(np.asarray(inputs["lif_w"], dtype=np.float32))
    tau_inv = float(np.float32(1.0 / (1.0 + math.exp(-lif_w))))
    nc = _get_program(tau_inv)
    in_maps = _make_in_maps(inputs)
    res = run_bass_kernel_spmd(nc, in_maps, core_ids=list(range(NCORES)))
    kernel.last_results = res

    S = np.stack([r["sp"] for r in res.results]).reshape(
        NCORES, 2, 128, T, BL, N)
    out = np.transpose(S, (3, 0, 4, 5, 1, 2))
    # sp holds the inverted spike s' = (U < th); emit 1 - s'
    return (1 - np.ascontiguousarray(out).reshape(T, B, N, C)).astype(
        np.float32)


# revision 28
# speedup vs baseline: 1.0471x; 1.0471x over previous
"""Trainium2 Bass kernel for nn_ARIG_Fusion (dual sigmoid gating + proj + BatchNorm + LIF).

Strategy (8 NeuronCores, SPMD):
  - Shard batch B=32 into 8 shards of 4. Each core handles rows (t, b_loc, n)
    = 4*4*1024 = 16384 rows of C=256 channels.
  - All tensors live on-chip in TRANSPOSED layout [channel, row]; the host
    pre-transposes inputs and post-transposes outputs.
  - Inputs stay f32 in HBM/SBUF but are DECLARED f32r, so the gate GEMMs
    consume them directly (f32r = same bits, 1 cyc/row on the PE vs 4 for
    fp32). The projection GEMM also runs f32r.
  - b_proj is dropped on device: BatchNorm subtracts the per-channel mean,
    so a per-channel bias before BN cancels exactly.
  - outb is stored as int16 (Act Copy with scale=QS): absolute quantization
    error (1/2QS ~ 4e-5) beats fp16's relative error ~8x and halves SBUF.
  - BatchNorm stats: per-tile bn_stats on the int16 outb -> bn_aggr, then a
    [128,4] AllReduce across 8 cores combines (mean, E[x^2]) in quant units.
  - The stats barrier serializes two phases, so each phase is balanced
    across engines. Engine facts (cost model): Pool elementwise is Q7
    software at ~2 ns/row + 95 ns/op and supports only TensorTensor and
    memset (no TensorScalarPtr); DVE TT is ~1.09 ns/row, DVE tensor_scalar
    is ~0.55 (2x mode, all-SBUF); ACT is ~0.89 ns/row incl. overhead.
  - Gating: v1 on DVE; v2/vr-add split DVE/Pool by a column fraction;
    bn_stats on DVE; outb copy split ACT/DVE. The ~33 MB input DMA is the
    phase floor.
  - LIF (T=4, rescaled state U_t = v_t/(1-tau)^t): per (t, q-chunk, j-half)
    ACT dequantizes + applies the BN affine (ay = asc_t*outb + ash_t), then
    the (q,j) recurrence chain runs ENGINE-LOCAL (no cross-engine hops in
    the scan): U = ay + W; s' = (U < th_t) u8 (inverted spike -> the host
    emits 1 - s'); W = U * s' (hard reset). 6 chains on DVE (immediate
    thresholds, is_lt in 2x mode) + 2 chains on Pool (is_lt as TT against
    memset threshold tiles).
"""

import math

import numpy as np

T, B, N, C = 4, 32, 1024, 256
NCORES = 8
BL = B // NCORES          # 4 batches per core
R = T * BL * N            # 16384 rows per core
RT = BL * N               # 4096 rows per t-slice
F = 512                   # gating tile columns
NT = R // F               # 32 gating tiles
CC = 1024                 # LIF column chunk (nq = RT // CC)
EPS = 1e-5
V_TH = 1.0
QS = 12288.0              # outb int16 quantization scale (range ~±2.67)

_program_cache = {}

# tuning knobs (read at trace time)
GIN_BUFS = 3
GATE_BUFS = 2
V_BUFS = 2
PG_BUFS = 1
PO_BUFS = 2
LIF_BUFS = 3
AY_BUFS = 4
S_BUFS = 12
STAGGER = 1              # software-pipeline proj/copy/bn by 1-2 tiles
V2_POOL_COLS = 1024      # cols of v2 (of 2*F=1024) computed on Pool
ADD_POOL_COLS = 256       # cols of vr add computed on Pool
COPY_DVE_COLS = 0        # cols of outb copy (of 1024) done on DVE
COPY_POOL_COLS = 0       # cols of outb copy done on Pool (TT vs QS tile)
N_POOL_HALVES = 7        # LIF chains (of 16) whose add/mult run on Pool
IN_DMA_QUEUES = 1        # spread xt input DMAs across sync/ACT queues


def _build_program(tau_inv: float, reps: int = 1, single_core: bool = False):
    cc = CC
    nq = RT // cc
    import concourse.bacc as bacc
    import concourse.bass as bass
    import concourse.tile as tile
    from concourse import mybir

    f32 = mybir.dt.float32
    f32r = mybir.dt.float32r
    i16 = mybir.dt.int16
    u8 = mybir.dt.uint8
    Alu = mybir.AluOpType
    Act = mybir.ActivationFunctionType

    # LIF rescaling: U_t = v_t / (1-tau)^t;  U_t = W_{t-1} + alpha_t * y_t
    one_m = 1.0 - tau_inv
    alphas = [tau_inv / (one_m ** t) for t in range(T)]
    ths = [V_TH / (one_m ** t) for t in range(T)]

    # LIF chains: state is elementwise, so the (q, j) scan splits freely by
    # column range into uniform half-chains of hc columns; the last
    # N_POOL_HALVES run their compare/mult on Pool, the rest on DVE.
    hc = cc // 2
    qjs = [(q, j) for j in (0, 1) for q in range(nq)]
    nchain = 2 * len(qjs)

    nc = bacc.Bacc("TRN2", target_bir_lowering=False, debug=False,
                   num_devices=1 if single_core else NCORES)

    xt_d = nc.dram_tensor("xt", [2, 2, 128, R], f32r, kind="ExternalInput")
    w1_d = nc.dram_tensor("w1", [128, 2, 2, 128], f32r, kind="ExternalInput")
    w2_d = nc.dram_tensor("w2", [128, 2, 2, 128], f32r, kind="ExternalInput")
    w3_d = nc.dram_tensor("w3", [128, 1, 2, 2, 128], f32r,
                          kind="ExternalInput")
    pp_d = nc.dram_tensor("pp", [128, 10], f32, kind="ExternalInput")
    bm_d = nc.dram_tensor("bm", [1, 2, 2, 128], f32r, kind="ExternalInput")
    on_d = nc.dram_tensor("on", [1, F], f32r, kind="ExternalInput")
    sp_d = nc.dram_tensor("sp", [2, 128, R], u8, kind="ExternalOutput")

    with tile.TileContext(nc) as tc:
      for _rep in range(reps):
        with tc.tile_pool(name="singles", bufs=1) as singles:
            w1s = singles.tile([128, 2, 2, 128], f32r)
            w2s = singles.tile([128, 2, 2, 128], f32r)
            w3s = singles.tile([128, 1, 2, 2, 128], f32r)
            pps = singles.tile([128, 10], f32)
            outb = singles.tile([128, 2, R], i16)
            stb = singles.tile([128, 2, NT, 6], f32)
            nc.sync.dma_start(w1s[:], w1_d[:, :, :, :])
            nc.sync.dma_start(w2s[:], w2_d[:, :, :, :])
            nc.sync.dma_start(w3s[:], w3_d[:, :, :, :, :])
            nc.sync.dma_start(pps[:], pp_d[:, :])
            bms = singles.tile([1, 2, 2, 128], f32r)
            ons = singles.tile([1, F], f32r)
            nc.sync.dma_start(bms[:], bm_d[:, :, :, :])
            nc.sync.dma_start(ons[:], on_d[:, :])
            if COPY_POOL_COLS > 0:
                qst = singles.tile([128, COPY_POOL_COLS], f32)
                nc.gpsimd.memset(qst[:], QS)

            # ---------------- gating + projection + stats ----------------
            with (
                tc.tile_pool(name="gin", bufs=GIN_BUFS) as gin,
                tc.tile_pool(name="gate", bufs=GATE_BUFS) as gatep,
                tc.tile_pool(name="vp", bufs=V_BUFS) as vp,
                tc.tile_pool(name="pg", bufs=PG_BUFS, space="PSUM") as pg,
                tc.tile_pool(name="po", bufs=PO_BUFS, space="PSUM") as po,
            ):
                # Software-pipelined: proj matmuls, the outb copy, and
                # bn_stats for tile k are emitted in LATER iterations so no
                # engine head-of-line blocks on a same-tile dependency.
                def emit_proj(k, vrk):
                    p3 = po.tile([128, 2, F], f32, tag="o", name="po")
                    for j in (0, 1):
                        nc.tensor.matmul(p3[:, j, :], w3s[:, 0, 0, j, :],
                                         vrk[:, 0, :], start=True, stop=False)
                        nc.tensor.matmul(p3[:, j, :], w3s[:, 0, 1, j, :],
                                         vrk[:, 1, :], start=False, stop=True)
                    return p3

                def emit_copy(k, p3k):
                    # quantize to int16 (scale QS); b_proj dropped (BN cancels)
                    # 3-way col split: ACT (Act.Copy), DVE (tensor_scalar),
                    # Pool (TT mult against the QS-memset tile)
                    cd, cp = COPY_DVE_COLS, COPY_POOL_COLS
                    ca = 2 * F - cd - cp
                    assert ca >= F, "ACT must cover at least the j=0 half"
                    k0 = k * F
                    nc.scalar.activation(outb[:, 0, k0:k0 + F], p3k[:, 0, :],
                                         Act.Copy, scale=QS)
                    e1 = ca - F   # ACT cols within j=1
                    if e1 > 0:
                        nc.scalar.activation(outb[:, 1, k0:k0 + e1],
                                             p3k[:, 1, :e1], Act.Copy,
                                             scale=QS)
                    if cd > 0:
                        nc.vector.tensor_scalar(outb[:, 1, k0 + e1:k0 + e1 + cd],
                                                p3k[:, 1, e1:e1 + cd], QS,
                                                None, Alu.mult)
                    if cp > 0:
                        nc.gpsimd.tensor_mul(outb[:, 1, k0 + e1 + cd:k0 + F],
                                             p3k[:, 1, e1 + cd:], qst[:, :cp])

                def emit_bn(k):
                    slk = slice(k * F, (k + 1) * F)
                    for j in (0, 1):
                        nc.vector.bn_stats(stb[:, j, k, :], outb[:, j, slk])

                vr_q = [None, None]   # vr tiles awaiting proj
                p3_q = [None, None]   # p3 tiles awaiting copy
                dma_engs = ([nc.sync, nc.scalar] if IN_DMA_QUEUES == 2
                            else [nc.sync])
                for i in range(NT):
                    sl = slice(i * F, (i + 1) * F)
                    xt = gin.tile([128, 2, 2, F], f32r, tag="xt", name="xt")
                    dap = xt_d[:, :, :, sl]
                    dma_engs[i % len(dma_engs)].dma_start(
                        xt[:], bass.AP(tensor=dap.tensor,
                                       offset=dap.offset,
                                       ap=[dap.ap[2], dap.ap[0],
                                           dap.ap[1], dap.ap[3]]))
                    a2 = xt[:, 0]
                    l2 = xt[:, 1]
                    g1 = pg.tile([128, 2, F], f32, tag="g1", name="g1")
                    g2 = pg.tile([128, 2, F], f32, tag="g2", name="g2")
                    for j in (0, 1):
                        nc.tensor.matmul(g1[:, j, :], bms[:, 0, j, :],
                                         ons[:, :], start=True, stop=False)
                        nc.tensor.matmul(g1[:, j, :], w1s[:, 0, j, :],
                                         a2[:, 0, :], start=False, stop=False)
                        nc.tensor.matmul(g1[:, j, :], w1s[:, 1, j, :],
                                         a2[:, 1, :], start=False, stop=True)
                    if STAGGER and i > 0:
                        p3_q[1] = emit_proj(i - 1, vr_q[1])
                    for j in (0, 1):
                        nc.tensor.matmul(g2[:, j, :], bms[:, 1, j, :],
                                         ons[:, :], start=True, stop=False)
                        nc.tensor.matmul(g2[:, j, :], w2s[:, 0, j, :],
                                         l2[:, 0, :], start=False, stop=False)
                        nc.tensor.matmul(g2[:, j, :], w2s[:, 1, j, :],
                                         l2[:, 1, :], start=False, stop=True)
                    gL = gatep.tile([128, 2, F], f32, tag="gL", name="gL")
                    gA = gatep.tile([128, 2, F], f32, tag="gA", name="gA")
                    nc.scalar.activation(gL[:], g1[:], Act.Sigmoid)
                    nc.scalar.activation(gA[:], g2[:], Act.Sigmoid)
                    if STAGGER and i > 0:
                        emit_copy(i - 1, p3_q[1])
                    if STAGGER and i > 1:
                        emit_bn(i - 2)
                    v1 = vp.tile([128, 2, F], f32, tag="v1", name="v1")
                    v2 = vp.tile([128, 2, F], f32, tag="v2", name="v2")
                    vr = vp.tile([128, 2, F], f32r, tag="vr", name="vr")
                    a2f = a2.bitcast(f32)
                    l2f = l2.bitcast(f32)
                    nc.vector.tensor_mul(v1[:], a2f, gA[:])
                    # col-split v2 / add across DVE and Pool (per j half)
                    vp_c = V2_POOL_COLS // 2
                    if vp_c > 0:
                        nc.gpsimd.tensor_mul(v2[:, :, F - vp_c:],
                                             l2f[:, :, F - vp_c:],
                                             gL[:, :, F - vp_c:])
                    if vp_c < F:
                        nc.vector.tensor_mul(v2[:, :, :F - vp_c],
                                             l2f[:, :, :F - vp_c],
                                             gL[:, :, :F - vp_c])
                    ap_c = ADD_POOL_COLS // 2
                    if ap_c > 0:
                        nc.gpsimd.tensor_add(vr[:, :, F - ap_c:],
                                             v1[:, :, F - ap_c:],
                                             v2[:, :, F - ap_c:])
                    if ap_c < F:
                        nc.vector.tensor_add(vr[:, :, :F - ap_c],
                                             v1[:, :, :F - ap_c],
                                             v2[:, :, :F - ap_c])
                    vr_q[1] = vr
                    if not STAGGER:
                        p3 = emit_proj(i, vr_q[1])
                        emit_copy(i, p3)
                        emit_bn(i)
                if STAGGER:
                    # drain: last tile's proj/copy/bn
                    p3_last = emit_proj(NT - 1, vr_q[1])
                    emit_copy(NT - 1, p3_last)
                    emit_bn(NT - 2)
                    emit_bn(NT - 1)
            # ---------------- stats finalize + all-reduce ----------------
            with (
                tc.tile_pool(name="fin", bufs=1) as fin,
                tc.tile_pool(name="dramp", bufs=1, space="DRAM") as dramp,
            ):
                mv = fin.tile([128, 2, 2], f32)
                ccs = fin.tile([128, 4], f32)
                for j in (0, 1):
                    nc.vector.bn_aggr(mv[:, j, :], stb[:, j, :, :])
                    nc.vector.tensor_copy(ccs[:, 2 * j:2 * j + 1], mv[:, j, 0:1])
                    nc.vector.tensor_mul(ccs[:, 2 * j + 1:2 * j + 2],
                                         mv[:, j, 0:1], mv[:, j, 0:1])
                    nc.vector.tensor_add(ccs[:, 2 * j + 1:2 * j + 2],
                                         ccs[:, 2 * j + 1:2 * j + 2],
                                         mv[:, j, 1:2])
                if single_core:
                    cg = fin.tile([128, 4], f32)
                    nc.vector.tensor_scalar(cg[:], ccs[:], float(NCORES),
                                            None, Alu.mult)
                else:
                    cc_in = dramp.tile([128, 4], f32)
                    cc_out = dramp.tile([128, 4], f32)
                    nc.gpsimd.dma_start(cc_in[:], ccs[:])
                    nc.gpsimd.collective_compute(
                        "AllReduce", Alu.add,
                        replica_groups=[list(range(NCORES))],
                        ins=[cc_in.opt()], outs=[cc_out.opt()],
                    )
                    cg = fin.tile([128, 4], f32)
                    nc.gpsimd.dma_start(cg[:], cc_out[:])

                mean = fin.tile([128, 2], f32)
                varp = fin.tile([128, 2], f32)
                sc = fin.tile([128, 2], f32)
                sh = fin.tile([128, 2], f32)
                t1 = fin.tile([128, 2], f32)
                t2 = fin.tile([128, 2], f32)
                r0 = fin.tile([128, 2], f32)
                for j in (0, 1):
                    jm = slice(j, j + 1)
                    nc.vector.tensor_scalar(mean[:, jm], cg[:, 2 * j:2 * j + 1],
                                            1.0 / NCORES, None, Alu.mult)
                    # varp = E[x^2] - mean^2 + eps  (in quant units: eps*QS^2)
                    nc.vector.tensor_scalar(varp[:, jm],
                                            cg[:, 2 * j + 1:2 * j + 2],
                                            1.0 / NCORES, None, Alu.mult)
                    nc.vector.tensor_mul(t1[:, jm], mean[:, jm], mean[:, jm])
                    nc.vector.tensor_sub(varp[:, jm], varp[:, jm], t1[:, jm])
                    nc.vector.tensor_scalar(varp[:, jm], varp[:, jm],
                                            EPS * QS * QS, None, Alu.add)
                # r0 = 1/sqrt(varp), via ACT sqrt + reciprocal + 2 Newton steps
                nc.scalar.activation(r0[:], varp[:], Act.Sqrt)
                nc.vector.reciprocal(r0[:], r0[:])
                for _ in range(2):
                    nc.vector.tensor_mul(t1[:], r0[:], r0[:])
                    nc.vector.tensor_mul(t2[:], t1[:], varp[:])
                    nc.vector.tensor_scalar(t2[:], t2[:], -0.5, 1.5,
                                            Alu.mult, Alu.add)
                    nc.vector.tensor_mul(r0[:], r0[:], t2[:])
                # sc = gamma * r0 (y = sc*outb_q + sh with outb_q in quant
                # units; the QS scale folds into r0 automatically)
                for j in (0, 1):
                    jm = slice(j, j + 1)
                    nc.vector.tensor_mul(sc[:, jm], pps[:, 6 + j:7 + j],
                                         r0[:, jm])
                    nc.vector.tensor_mul(t1[:, jm], mean[:, jm], sc[:, jm])
                    nc.vector.tensor_sub(sh[:, jm], pps[:, 8 + j:9 + j],
                                         t1[:, jm])

                # per-t pre-scaled BN affine: ay_t = alpha_t*(sc*outb+sh)
                asc = fin.tile([128, T, 2], f32)
                ash = fin.tile([128, T, 2], f32)
                for t in range(T):
                    for j in (0, 1):
                        nc.vector.tensor_scalar(asc[:, t, j:j + 1], sc[:, j:j + 1],
                                                alphas[t], None, Alu.mult)
                        nc.vector.tensor_scalar(ash[:, t, j:j + 1], sh[:, j:j + 1],
                                                alphas[t], None, Alu.mult)

                # ---------------- LIF scan + spike output ----------------
                # s' = (U < th) is the INVERTED spike: W = U*s' gives the
                # hard reset, and the host emits spikes = 1 - s'.
                # Each chain (q, j, col-range) runs its compare/mult on ONE
                # engine; with LIF_PE_ADD the U=ay+W add runs on the (idle)
                # PE as two identity matmuls accumulating into PSUM.
                # t-outer, op-type-grouped: per t-row each engine emits
                # all its adds, then compares, then resets - consecutive ops
                # on an engine belong to different chains, so dependent-op
                # latency is hidden. ALL compares run on DVE (tensor_scalar
                # u8-out is 2x there; Pool cannot emit u8 from f32): Pool
                # owns add/mult for the last N_POOL_HALVES chains (f32 TT).
                # State lives IN-PLACE in the ay tile: ay += W_prev;
                # s' = (ay < th) [DVE]; ay *= s' (ay then IS W for t+1).
                # Pool-owned chains' compares are emitted first so Pool's
                # resets unblock early.
                pool_own = set(range(nchain - N_POOL_HALVES, nchain))

                def chain_slice(ci, t):
                    qi, half = divmod(ci, 2)
                    q, j = qjs[qi]
                    c0 = t * RT + q * cc + half * hc
                    return j, slice(c0, c0 + hc)

                def ceng(ci):
                    return nc.gpsimd if ci in pool_own else nc.vector

                cmp_order = (list(range(nchain - N_POOL_HALVES, nchain))
                             + list(range(nchain - N_POOL_HALVES)))
                with tc.tile_pool(name="lif", bufs=LIF_BUFS) as lifp:
                    ayt = {}
                    prev = {}
                    for t in range(T):
                        for ci in range(nchain):
                            en = "p" if ci in pool_own else "d"
                            nch = (N_POOL_HALVES if ci in pool_own
                                   else nchain - N_POOL_HALVES)
                            j, csl = chain_slice(ci, t)
                            ay = lifp.tile([128, hc], f32, tag=f"ay{en}",
                                           name=f"ay{en}", bufs=2 * nch + 2)
                            nc.scalar.activation(
                                ay[:], outb[:, j, csl], Act.Identity,
                                bias=ash[:, t, j:j + 1],
                                scale=asc[:, t, j:j + 1])
                            prev[ci] = ayt.get(ci)
                            ayt[ci] = ay
                        if t > 0:
                            for ci in list(range(nchain - N_POOL_HALVES,
                                                 nchain))                                     + list(range(nchain - N_POOL_HALVES)):
                                ceng(ci).tensor_add(ayt[ci][:], ayt[ci][:],
                                                    prev[ci][:])
                        st = {}
                        for ci in cmp_order:
                            s = lifp.tile([128, hc], u8, tag="s", name="s",
                                          bufs=S_BUFS)
                            nc.vector.tensor_scalar(s[:], ayt[ci][:],
                                                    float(ths[t]), None,
                                                    Alu.is_lt)
                            st[ci] = s
                            j, csl = chain_slice(ci, t)
                            nc.sync.dma_start(sp_d[j, :, csl], s[:])
                        if t < T - 1:
                            for ci in cmp_order:
                                ceng(ci).tensor_mul(ayt[ci][:], ayt[ci][:],
                                                    st[ci][:])

    nc.compile()
    return nc


def _get_program(tau_inv: float, reps: int = 1, single_core: bool = False):
    key = (round(float(tau_inv), 12), reps, single_core)
    if key not in _program_cache:
        _program_cache[key] = _build_program(float(tau_inv), reps, single_core)
    return _program_cache[key]


def _shard_transpose(x):
    # [T,B,N,C] -> [cores, 2, 128, R] with rows ordered (t, b_loc, n)
    v = x.reshape(T, NCORES, BL, N, C)
    v = np.transpose(v, (1, 4, 0, 2, 3))
    return np.ascontiguousarray(v).reshape(NCORES, 2, 128, R)


def _prep_w(w):
    # lhsT chunks [p, k, j, q]: W.T viewed as [k,128p][j,128q]
    wt = np.ascontiguousarray(w.T).reshape(2, 128, 2, 128)
    return np.ascontiguousarray(wt.transpose(1, 0, 2, 3))


def _two(vec):
    return np.ascontiguousarray(vec.reshape(2, 128).T)


def _make_in_maps(inputs):
    x_attn = np.asarray(inputs["x_attn"], dtype=np.float32)
    x_lsm = np.asarray(inputs["x_lsm"], dtype=np.float32)
    at = _shard_transpose(x_attn)
    lt = _shard_transpose(x_lsm)
    xt = np.stack([at, lt], axis=1)  # [cores, 2, 2, 128, R]
    w1 = _prep_w(np.asarray(inputs["W_att"], dtype=np.float32))
    w2 = _prep_w(np.asarray(inputs["W_lsm"], dtype=np.float32))
    w3 = _prep_w(np.asarray(inputs["W_proj"], dtype=np.float32))[:, None]
    pp = np.concatenate(
        [_two(np.asarray(inputs["b_att"], dtype=np.float32)),
         _two(np.asarray(inputs["b_lsm"], dtype=np.float32)),
         _two(np.asarray(inputs["b_proj"], dtype=np.float32)),
         _two(np.asarray(inputs["gamma"], dtype=np.float32)),
         _two(np.asarray(inputs["beta"], dtype=np.float32))],
        axis=1)
    bm = np.ascontiguousarray(np.stack([
        np.asarray(inputs["b_att"], dtype=np.float32).reshape(2, 128),
        np.asarray(inputs["b_lsm"], dtype=np.float32).reshape(2, 128),
    ])[None])  # [1, 2, 2, 128]
    base = {"w1": w1, "w2": w2, "w3": w3, "pp": pp, "bm": bm,
            "on": np.ones((1, F), dtype=np.float32)}
    return [dict(base, xt=xt[s]) for s in range(NCORES)]


def kernel(**inputs):
    from concourse.bass_utils import run_bass_kernel_spmd

    lif_w = float(np.asarray(inputs["lif_w"], dtype=np.float32))
    tau_inv = float(np.float32(1.0 / (1.0 + math.exp(-lif_w))))
    nc = _get_program(tau_inv)
    in_maps = _make_in_maps(inputs)
    res = run_bass_kernel_spmd(nc, in_maps, core_ids=list(range(NCORES)))
    kernel.last_results = res

    S = np.stack([r["sp"] for r in res.results]).reshape(
        NCORES, 2, 128, T, BL, N)
    out = np.transpose(S, (3, 0, 4, 5, 1, 2))
    # sp holds the inverted spike s' = (U < th); emit 1 - s'
    return (1 - np.ascontiguousarray(out).reshape(T, B, N, C)).astype(
        np.float32)
